# revision 20
# baseline (speedup 1.0000x reference)
"""Trainium2 Bass kernel for nn_EntropyModel (MoE routing over K=4 class towers).

Strategy: every op in the tower is a per-pixel 1x1 conv (matmul over channels),
and the final one-hot masked sum selects exactly one class tower per pixel.
So route on the host: sort pixels by seg class, give each of the 8 cores half
of one class's pixels (expert-parallel, 2 cores per class), run that class's
tower densely on its gathered pixels, and scatter the results back.

The 5-matmul tower is algebraically collapsed to 4 matmuls per pixel by
folding the linear layers around the two LeakyReLUs (host precomputes the
merged 128x128 weights):
    a2 = lrelu(V x + c)          V  = Wr1 W1,      c   = Wr1 b1 + br1
    h3 = lrelu(T x + U a2 + b3') T  = W3 W1,       U   = W3 Wr2,
                                 b3' = W3 (b1 + br2) + b3
    y  = W4 h3 + b4
Matmuls run in float32r (reduced-precision fp32 PE mode, ~1e-4 rel err per
matmul, 4x faster than full fp32).
"""
import numpy as np

import concourse.mybir as mybir
import concourse.tile as tile
from concourse import bacc
from concourse.bass_utils import run_bass_kernel_spmd

B, C, H, W = 2, 128, 192, 192
K = 4
O = 60
NTOT = B * H * W
NCORES = 8
MACRO = 1024  # free-dim per ACT/PSUM chunk (2 PSUM banks)
MMF = 512     # free-dim per matmul (1 PSUM bank, fp32)

F32 = mybir.dt.float32
F32R = mybir.dt.float32r

LAST_RESULTS = None  # test harness reads exec_time_ns off this

_nc_cache = {}


def _build(cap):
    nc = bacc.Bacc(None, target_bir_lowering=False)
    x = nc.dram_tensor("x", [C, cap], F32R, kind="ExternalInput")
    # packed weights: [vt | tt | ut | w4t] along free dim
    wp = nc.dram_tensor("wp", [C, 3 * C + O], F32R, kind="ExternalInput")
    # packed biases: [c | b3' | b4(rows 0..59)]
    bp = nc.dram_tensor("bp", [C, 3], F32, kind="ExternalInput")
    y = nc.dram_tensor("y", [O, cap], F32, kind="ExternalOutput")

    # compute chunks: small first chunk to start the pipeline early, small
    # last chunk to shorten the serial wind-down
    spans = []
    s = 0
    while s < cap:
        rem = cap - s
        if (s == 0 or rem <= MMF) and cap > 2 * MACRO:
            w = MMF
        elif rem > MACRO:
            w = MACRO
        else:
            w = rem if (rem <= MMF or cap <= 2 * MACRO) else rem - MMF
        spans.append((s, w))
        s += w

    Lrelu = mybir.ActivationFunctionType.Lrelu

    # Single integrated pipeline: per 1024-col chunk c the PE runs
    # V(c), T(c), U(c) and W4(c-1) back-to-back while ACT computes the two
    # LeakyReLUs and DVE the final bias-copy one chunk behind — so the x
    # stream, PE, ACT, DVE and the y stream all overlap. 4 PSUM slots of
    # 2 banks each give the rotation enough slack that PE's in-order
    # stream never waits on ACT. Intermediates live full-size in SBUF.
    with tile.TileContext(nc) as tc:
        with tc.tile_pool(name="const", bufs=1) as cw, \
             tc.tile_pool(name="big", bufs=1) as bigp, \
             tc.tile_pool(name="ps", bufs=4, space="PSUM") as ps:
            xt = bigp.tile([C, cap], F32R)
            a2t = bigp.tile([C, cap], F32R)
            h3t = bigp.tile([C, cap], F32R)
            yt = bigp.tile([O, cap], F32)

            # weights first (small, needed by the first matmul), then x:
            # small leading slabs so the first compute chunks unblock early,
            # then 2048-col slabs (decoupled from the compute chunking).
            # weights ride the (otherwise idle) GpSimd SWDGE channel so they
            # stream concurrently with sync's x slabs
            wpt = cw.tile([C, 3 * C + O], F32R)
            nc.gpsimd.dma_start(wpt[:], wp[:])
            bpt = cw.tile([C, 3], F32)
            nc.gpsimd.dma_start(bpt[:], bp[:])
            s = 0
            for slab in (512, 1536):
                w = min(slab, cap - s)
                nc.sync.dma_start(xt[:, s:s + w], x[:, s:s + w])
                s += w
                if s >= cap:
                    break
            while s < cap:
                w = min(2048, cap - s)
                nc.sync.dma_start(xt[:, s:s + w], x[:, s:s + w])
                s += w

            vtt = wpt[:, 0:C]
            ttt = wpt[:, C:2 * C]
            utt = wpt[:, 2 * C:3 * C]
            w4tt = wpt[:, 3 * C:3 * C + O]
            cbt = bpt[:, 0:1]
            b3t = bpt[:, 1:2]
            b4t = bpt[:O, 2:3]

            def w4_stage(s, w):
                py = ps.tile([O, MACRO], F32, tag="mm", name="py")[:, :w]
                for j in range(s, s + w, MMF):
                    nc.tensor.matmul(py[:, j - s:j - s + MMF], w4tt,
                                     h3t[:, j:j + MMF], start=True, stop=True)
                nc.vector.tensor_scalar_add(yt[:, s:s + w], py[:], b4t)

            # ACT is the pacing engine (2 LeakyReLUs/chunk) while DVE only does
            # the final bias-copy; shift the a2-lrelu of a couple of middle
            # chunks to DVE (3-op max(z, 0.01z)) to balance the two.
            dve_lrelu_cis = {3, 6} if len(spans) >= 9 else set()

            ydone = 0  # columns of y stored so far
            for ci, (s, w) in enumerate(spans):
                pa = ps.tile([C, MACRO], F32, tag="mm", name="pa")[:, :w]
                for j in range(s, s + w, MMF):
                    nc.tensor.matmul(pa[:, j - s:j - s + MMF], vtt,
                                     xt[:, j:j + MMF], start=True, stop=True)
                if ci in dve_lrelu_cis:
                    # lrelu(z+b) = max(z+b, 0.01*(z+b)) in two DVE ops:
                    #   zq = (pa + cb) * 0.01 ; a2 = (pa + cb) max zq
                    zq = bigp.tile([C, MACRO], F32R, tag="zq", name="zq")[:, :w]
                    nc.vector.tensor_scalar(zq[:], pa[:], cbt, 0.01,
                                            mybir.AluOpType.add,
                                            mybir.AluOpType.mult)
                    nc.vector.scalar_tensor_tensor(
                        a2t[:, s:s + w], pa[:], cbt, zq[:],
                        mybir.AluOpType.add, mybir.AluOpType.max)
                else:
                    nc.scalar.activation(a2t[:, s:s + w], pa[:], Lrelu,
                                         bias=cbt, scale=1.0, alpha=0.01)

                ph = ps.tile([C, MACRO], F32, tag="mm", name="ph")[:, :w]
                for j in range(s, s + w, MMF):
                    nc.tensor.matmul(ph[:, j - s:j - s + MMF], ttt,
                                     xt[:, j:j + MMF], start=True, stop=False)
                for j in range(s, s + w, MMF):
                    nc.tensor.matmul(ph[:, j - s:j - s + MMF], utt,
                                     a2t[:, j:j + MMF], start=False, stop=True)
                nc.scalar.activation(h3t[:, s:s + w], ph[:], Lrelu,
                                     bias=b3t, scale=1.0, alpha=0.01)

                if ci > 0:
                    ps_, w_ = spans[ci - 1]
                    w4_stage(ps_, w_)
                    if ps_ + w_ - ydone >= 2048:
                        nc.sync.dma_start(y[:, ydone:ps_ + w_],
                                          yt[:, ydone:ps_ + w_])
                        ydone = ps_ + w_
            w4_stage(*spans[-1])
            nc.sync.dma_start(y[:, ydone:cap], yt[:, ydone:cap])
    nc.compile()
    return nc


def kernel(fusion_context, seg, W1, b1, Wr1, br1, Wr2, br2, W3, b3, W4, b4):
    global LAST_RESULTS
    fusion_context = np.asarray(fusion_context, dtype=np.float32)
    seg = np.asarray(seg)

    # [B,C,H,W] -> [C, B*H*W]; column n = (b, h, w) row-major
    xcols = np.ascontiguousarray(
        fusion_context.transpose(1, 0, 2, 3).reshape(C, NTOT))
    segf = seg.reshape(-1).astype(np.int64)

    # Route: per class index list, split into two halves -> 8 core shards
    shards = []  # (class_id, column_indices)
    for k in range(K):
        ix = np.nonzero(segf == k)[0]
        h = (len(ix) + 1) // 2
        shards.append((k, ix[:h]))
        shards.append((k, ix[h:]))
    assert len(shards) == NCORES

    cap = max(len(ix) for _, ix in shards)
    cap = max(MMF, -(-cap // MMF) * MMF)  # round up to matmul tile

    if cap not in _nc_cache:
        _nc_cache[cap] = _build(cap)
    nc = _nc_cache[cap]

    f64 = np.float64
    in_maps = []
    for k, ix in shards:
        xs = np.zeros((C, cap), dtype=np.float32)
        xs[:, :len(ix)] = xcols[:, ix]
        V = W1[k].astype(f64).T @ Wr1[k].astype(f64).T    # (Wr1 W1)^T
        T = W1[k].astype(f64).T @ W3[k].astype(f64).T     # (W3 W1)^T
        U = Wr2[k].astype(f64).T @ W3[k].astype(f64).T    # (W3 Wr2)^T
        c = Wr1[k].astype(f64) @ b1[k].astype(f64) + br1[k].astype(f64)
        b3p = W3[k].astype(f64) @ (b1[k].astype(f64) + br2[k].astype(f64)) \
            + b3[k].astype(f64)
        wp = np.concatenate(
            [V, T, U, W4[k].T.astype(f64)], axis=1).astype(np.float32)
        bp = np.zeros((C, 3), dtype=np.float32)
        bp[:, 0] = c
        bp[:, 1] = b3p
        bp[:O, 2] = b4[k]
        in_maps.append({
            "x": xs,
            "wp": np.ascontiguousarray(wp),
            "bp": bp,
        })

    res = run_bass_kernel_spmd(nc, in_maps, core_ids=list(range(NCORES)))
    LAST_RESULTS = res

    out = np.empty((O, NTOT), dtype=np.float32)
    for (k, ix), r in zip(shards, res.results):
        out[:, ix] = r["y"][:, :len(ix)]
    return np.ascontiguousarray(
        out.reshape(O, B, H * W).transpose(1, 0, 2).reshape(B, O, H, W))
